# revision 2
# baseline (speedup 1.0000x reference)
"""Ternary-expert MLP (gate/up/silu/down) on 8 trn2 NeuronCores.

Data-parallel over tokens (512/core). Weights are PURE ternary {-1,0,+1}
stored in fp8e4 (exact); per-channel scales applied after the matmuls.

Mixed-precision schedule over the 5632 intermediate channels: channels are
permuted (descending quantization-error impact, estimated at runtime from a
token subsample) so each 128-channel tile is homogeneous, then per tile:
  tiles  0..19  gate/up from bf16 x      (16 plain matmuls per proj)
  tiles 20..43  gate/up from fp8 x       (8 fp8 DoubleRow matmuls per proj,
                                          adjacent k-tiles paired, K=256)
  tiles  0..17  hdn stored bf16          (down: plain matmul per tile)
  tiles 18..43  hdn stored fp8           (down: DoubleRow, tiles paired)
fp8 DoubleRow runs 2x MACs/cycle, so cheap tiles cost half. Error lands at
~1.5e-2 absmax-relative (tolerance 2e-2); PE cycles drop ~450us -> ~325us.
"""

import numpy as np
import ml_dtypes

HIDDEN = 2048
INTER = 5632
B, S = 2, 2048
T = B * S
NCORES = 8
TPC = T // NCORES          # 512 tokens per core
P = 128
KH = HIDDEN // P           # 16 hidden-dim k-tiles
NI = INTER // P            # 44 intermediate tiles

NGU_BF = 16                # tiles 0..15: gate/up consume bf16 x
NH_BF = 14                 # tiles 0..13: hdn kept in bf16
NH_F8 = NI - NH_BF         # 30 fp8-hdn tiles (even -> 15 DoubleRow pairs)

F8 = ml_dtypes.float8_e4m3
BF = ml_dtypes.bfloat16

_cache = {}


def _build_nc(kh=KH, ni=NI, tpc=TPC):
    import concourse.bacc as bacc
    import concourse.tile as tile
    from concourse import mybir

    f8 = mybir.dt.float8e4
    bf16 = mybir.dt.bfloat16
    f16 = mybir.dt.float16
    f32 = mybir.dt.float32
    DR = mybir.MatmulPerfMode.DoubleRow
    AF = mybir.ActivationFunctionType

    nc = bacc.Bacc("TRN2", target_bir_lowering=False, debug=False)
    xtb = nc.dram_tensor("xtb", [P, kh, tpc], bf16, kind="ExternalInput").ap()
    xt8 = nc.dram_tensor("xt8", [P, kh, tpc], f8, kind="ExternalInput").ap()
    gw = nc.dram_tensor("gw", [ni, P, kh * P], f8, kind="ExternalInput").ap()
    uw = nc.dram_tensor("uw", [ni, P, kh * P], f8, kind="ExternalInput").ap()
    dw = nc.dram_tensor("dw", [kh, P, ni * P], f8, kind="ExternalInput").ap()
    gs = nc.dram_tensor("gs", [P, ni], f32, kind="ExternalInput").ap()
    us = nc.dram_tensor("us", [P, ni], f32, kind="ExternalInput").ap()
    ds = nc.dram_tensor("ds", [P, kh], f32, kind="ExternalInput").ap()
    ot = nc.dram_tensor("ot", [kh, P, tpc], f32, kind="ExternalOutput").ap()

    with tile.TileContext(nc) as tc:
        with (
            tc.tile_pool(name="xp", bufs=1) as xp,
            tc.tile_pool(name="hp", bufs=1) as hp,
            tc.tile_pool(name="sp", bufs=1) as sp,
            tc.tile_pool(name="wg", bufs=3) as wg_pool,
            tc.tile_pool(name="wu", bufs=3) as wu_pool,
            tc.tile_pool(name="wd", bufs=3) as wd_pool,
            tc.tile_pool(name="act", bufs=3) as act_pool,
            tc.tile_pool(name="ob", bufs=3) as ob_pool,
            tc.tile_pool(name="ps", bufs=2, space="PSUM") as ps_pool,
            tc.tile_pool(name="po", bufs=2, space="PSUM") as po_pool,
        ):
            # PE warmup: HAM un-throttles only after ~3.4us of sustained PE
            # activity; bridge the initial DMA wait with dummy matmuls on a
            # zeroed tile so the real stream starts at 2.4 GHz.
            wz = act_pool.tile([P, P], f16, tag="warm")
            nc.vector.memset(wz[:], 0.0)
            pw = po_pool.tile([P, P], f32, tag="warmp")
            for _ in range(34):
                nc.tensor.matmul(pw[:], wz[:], wz[:], start=True, stop=True)

            xb = xp.tile([P, kh, tpc], bf16, tag="xb")
            x8 = xp.tile([P, kh, tpc], f8, tag="x8")
            hbf = hp.tile([P, NH_BF, tpc], bf16, tag="hbf")
            hf8 = hp.tile([P, NH_F8, tpc], f8, tag="hf8")
            gssb = sp.tile([P, ni], f32, tag="gs")
            ussb = sp.tile([P, ni], f32, tag="us")
            dssb = sp.tile([P, kh], f32, tag="ds")

            # fp8 tiles run first: they only need x8 (1MB), so the bf16 x
            # stream has the whole fp8 region (~95us) to land. First loads in
            # consumption order; bulk xb staged behind the early weight slabs.
            tile_order = list(range(NGU_BF, ni)) + list(range(NGU_BF))
            it0 = tile_order[0]
            wgt0 = wg_pool.tile([P, kh, P], f8, tag="wgt")
            nc.sync.dma_start(out=wgt0[:], in_=gw[it0])
            nc.sync.dma_start(out=x8[:, 0:4], in_=xt8[:, 0:4])
            wut0 = wu_pool.tile([P, kh, P], f8, tag="wut")
            nc.sync.dma_start(out=wut0[:], in_=uw[it0])
            nc.sync.dma_start(out=x8[:, 4:kh], in_=xt8[:, 4:kh])
            nc.sync.dma_start(out=gssb[:], in_=gs)
            nc.sync.dma_start(out=ussb[:], in_=us)

            for pos, it in enumerate(tile_order):
                if pos == 0:
                    wgt, wut = wgt0, wut0
                else:
                    wgt = wg_pool.tile([P, kh, P], f8, tag="wgt")
                    nc.sync.dma_start(out=wgt[:], in_=gw[it])
                    wut = wu_pool.tile([P, kh, P], f8, tag="wut")
                    nc.sync.dma_start(out=wut[:], in_=uw[it])
                if pos == 1:
                    nc.sync.dma_start(out=xb[:, 0:4], in_=xtb[:, 0:4])
                elif pos == 2:
                    nc.sync.dma_start(out=xb[:, 4:10], in_=xtb[:, 4:10])
                elif pos == 3:
                    nc.sync.dma_start(out=xb[:, 10:kh], in_=xtb[:, 10:kh])
                elif pos == 4:
                    nc.sync.dma_start(out=dssb[:], in_=ds)
                pg = ps_pool.tile([P, tpc], f32)
                pu = ps_pool.tile([P, tpc], f32)
                if it < NGU_BF:
                    for ph, w in ((pg, wgt), (pu, wut)):
                        for k in range(kh):
                            nc.tensor.matmul(
                                ph[:], w[:, k, :], xb[:, k, :],
                                start=(k == 0), stop=(k == kh - 1),
                            )
                else:
                    for ph, w in ((pg, wgt), (pu, wut)):
                        for j in range(kh // 2):
                            nc.tensor.matmul(
                                ph[:], w[:, 2 * j:2 * j + 2, :],
                                x8[:, 2 * j:2 * j + 2, :],
                                start=(j == 0), stop=(j == kh // 2 - 1),
                                perf_mode=DR,
                            )
                sl = act_pool.tile([P, tpc], f16, tag="sl")
                nc.scalar.activation(sl[:], pg[:], AF.Silu,
                                     scale=gssb[:, it:it + 1])
                m1 = act_pool.tile([P, tpc], f16, tag="m1")
                nc.vector.tensor_mul(m1[:], sl[:], pu[:])
                if it < NH_BF:
                    nc.vector.tensor_scalar_mul(hbf[:, it], m1[:],
                                                ussb[:, it:it + 1])
                else:
                    nc.vector.tensor_scalar_mul(hf8[:, it - NH_BF], m1[:],
                                                ussb[:, it:it + 1])

            for hg in range(kh):
                wdt = wd_pool.tile([P, ni, P], f8)
                nc.sync.dma_start(out=wdt[:], in_=dw[hg])
                po = po_pool.tile([P, tpc], f32)
                for t in range(NH_BF):
                    nc.tensor.matmul(
                        po[:], wdt[:, t, :], hbf[:, t, :],
                        start=(t == 0), stop=False,
                    )
                for j in range(NH_F8 // 2):
                    nc.tensor.matmul(
                        po[:], wdt[:, NH_BF + 2 * j:NH_BF + 2 * j + 2, :],
                        hf8[:, 2 * j:2 * j + 2, :],
                        start=False, stop=(j == NH_F8 // 2 - 1),
                        perf_mode=DR,
                    )
                ob = ob_pool.tile([P, tpc], f32)
                nc.scalar.activation(ob[:], po[:], AF.Copy,
                                     scale=dssb[:, hg:hg + 1])
                nc.sync.dma_start(out=ot[hg], in_=ob[:])

    nc.compile()
    return nc


def _channel_perm(x, gate_w, up_w, gate_s, up_s, ntok=768):
    """Permutation of INTER channels, descending fp8-quantization impact,
    estimated on a token subsample. Deterministic."""
    xs = x[::max(1, x.shape[0] // ntok)][:ntok]
    xhi = xs.astype(F8).astype(np.float32)
    g = (xs @ gate_w.T) * gate_s
    u = (xs @ up_w.T) * up_s
    h = g / (1.0 + np.exp(-g)) * u
    gB = (xhi @ gate_w.T) * gate_s
    uB = (xhi @ up_w.T) * up_s
    hB = gB / (1.0 + np.exp(-gB)) * uB
    hq = hB.astype(F8).astype(np.float32)
    v_B = np.mean((hq - h) ** 2, axis=0)
    return np.argsort(-v_B, kind="stable")


def _pack_weights(gate_w, up_w, down_w):
    gw = gate_w.reshape(NI, P, KH, P)
    gw = np.ascontiguousarray(gw.transpose(0, 3, 2, 1)).astype(F8)
    uw = up_w.reshape(NI, P, KH, P)
    uw = np.ascontiguousarray(uw.transpose(0, 3, 2, 1)).astype(F8)
    dwp = down_w.reshape(KH, P, NI, P)
    dwp = np.ascontiguousarray(dwp.transpose(0, 3, 2, 1)).astype(F8)
    return (gw.reshape(NI, P, KH * P), uw.reshape(NI, P, KH * P),
            dwp.reshape(KH, P, NI * P))


def _pack_x(xf):
    # per-core x^T tiles: [p, k, t] = x_core[t, k*128+p], bf16 + fp8 copies
    outs = []
    for c in range(NCORES):
        xc = xf[c * TPC:(c + 1) * TPC].reshape(TPC, KH, P)
        xct = np.ascontiguousarray(xc.transpose(2, 1, 0))
        outs.append((xct.astype(BF), xct.astype(F8)))
    return outs


def _ensure_ntff_hook():
    """bass_utils' axon trace path imports antenv.axon_hooks, which is
    missing from this image; provide it (ctypes into libaxon_pjrt.so) so a
    BASS_TRACE=1 environment doesn't crash the run."""
    import sys
    try:
        import antenv.axon_hooks  # noqa: F401
        return
    except ImportError:
        pass
    import contextlib
    import ctypes
    import types

    def _make_hook():
        try:
            lib = ctypes.CDLL("/opt/axon/libaxon_pjrt.so")
            lib.axon_start_nrt_profile
        except Exception:
            return None
        lib.axon_start_nrt_profile.argtypes = [ctypes.POINTER(ctypes.c_int64),
                                               ctypes.c_size_t]
        lib.axon_start_nrt_profile.restype = ctypes.c_int64
        lib.axon_stop_nrt_profile.argtypes = [ctypes.c_char_p]
        lib.axon_stop_nrt_profile.restype = ctypes.c_int64

        @contextlib.contextmanager
        def _hook(output_dir, device_ids):
            import jax
            jax.devices()
            if device_ids:
                ids = (ctypes.c_int64 * len(device_ids))(*device_ids)
                rc = lib.axon_start_nrt_profile(ids, len(device_ids))
            else:
                rc = lib.axon_start_nrt_profile(None, 0)
            if rc != 0:
                raise RuntimeError(f"axon_start_nrt_profile rc={rc}")
            try:
                yield
            finally:
                lib.axon_stop_nrt_profile(str(output_dir).encode())

        return _hook

    mod = types.ModuleType("antenv.axon_hooks")
    _hook = _make_hook()
    mod.get_axon_ntff_profile_hook = lambda: _hook
    mod.set_axon_ntff_profile_hook = lambda h: None
    sys.modules["antenv.axon_hooks"] = mod


def _run(in_maps, trace=False, tmpdir=None, trace_cores=None):
    from concourse.bass_utils import run_bass_kernel_spmd

    _ensure_ntff_hook()
    if "nc" not in _cache:
        _cache["nc"] = _build_nc()
    return run_bass_kernel_spmd(
        _cache["nc"], in_maps, list(range(NCORES)), trace=trace, tmpdir=tmpdir,
        trace_cores=trace_cores,
    )


def make_in_maps(x, gate_w, up_w, down_w, gate_s, up_s, down_s):
    x = np.asarray(x, np.float32)
    gate_w = np.asarray(gate_w, np.float32)
    up_w = np.asarray(up_w, np.float32)
    down_w = np.asarray(down_w, np.float32)
    gate_s = np.asarray(gate_s, np.float32)
    up_s = np.asarray(up_s, np.float32)
    down_s = np.asarray(down_s, np.float32)

    xf = x.reshape(T, HIDDEN)
    perm = _channel_perm(xf, gate_w, up_w, gate_s, up_s)
    gate_w = gate_w[perm]
    up_w = up_w[perm]
    down_w = down_w[:, perm]
    gate_s = gate_s[perm]
    up_s = up_s[perm]

    gw, uw, dwp = _pack_weights(gate_w, up_w, down_w)
    gss = np.ascontiguousarray(gate_s.reshape(NI, P).T)
    uss = np.ascontiguousarray(up_s.reshape(NI, P).T)
    dss = np.ascontiguousarray(down_s.reshape(KH, P).T)
    xts = _pack_x(xf)
    return [{"xtb": xts[c][0], "xt8": xts[c][1], "gw": gw, "uw": uw,
             "dw": dwp, "gs": gss, "us": uss, "ds": dss}
            for c in range(NCORES)]


def unpack_out(results):
    # ot: (16, 128, 512) f32 per core; out_core[t, hg*128+p] = ot[hg, p, t]
    parts = []
    for c in range(NCORES):
        ot = results[c]["ot"]
        parts.append(ot.transpose(2, 0, 1).reshape(TPC, HIDDEN))
    return np.concatenate(parts, axis=0).reshape(B, S, HIDDEN)


def kernel(x, gate_w, up_w, down_w, gate_s, up_s, down_s):
    in_maps = make_in_maps(x, gate_w, up_w, down_w, gate_s, up_s, down_s)
    try:
        res = _run(in_maps)
    except Exception:
        # transient runtime errors (device hiccup) — one retry
        res = _run(in_maps)
    return unpack_out(res.results)
